# revision 12
# baseline (speedup 1.0000x reference)
"""Trainium2 Bass kernel for nn_ContextLearner (gnn_message_passing).

Per-graph transformer block over 1024 graphs x 100 nodes:
  emb gather -> LN1 -> QK -> masked softmax attention -> combine -> LN2 -> FC

Sharding: data-parallel, 128 graphs per core across 8 NeuronCores.
Embedding table + weights replicated per core. Gather happens on-device
via indirect DMA (SWDGE), batched 800 descriptors per instruction.

Key algebraic restructuring vs the straightforward lowering: the output
only needs wc[h,:] = am[h] @ y where y = (attn @ v) @ P^T and
am = colmean(attn_h).  Associating the chain as
  amT = em_h^T @ r_h            (colsums of normalized attn, via PE)
  Z_h' = (amT * r_h')^T @ em_h' (fold 1/rowsum of head h' into amT)
  U_h' = Z_h'^T-via-xn          (UT[c,(h',h)] = xn^T @ ZT)
  wc   = sum_h' U_h' @ G_h',  G_h' = Wv_h'^T P^T  (host-precomputed)
means V-projection, attn@v and proj never materialize per query row --
per graph the combine path is a handful of tiny matmuls instead of
three [100,512]-sized GEMMs.  LN2 is scale-invariant so the 1/N of the
attn mean and the magnitude of G are free.

Numerics: bf16 matmuls with fp32 PSUM accumulation; LN stats / softmax
in fp32. LN gains folded into adjacent matmul weights on host (exact).
rsqrt computed as Exp(-0.5*Ln(x)) to stay in one ACT table set.
"""

import numpy as np
import ml_dtypes

import concourse.bass as bass
import concourse.tile as tile
from concourse import mybir
from concourse.bass_utils import run_bass_kernel_spmd

F32 = mybir.dt.float32
BF16 = mybir.dt.bfloat16
I32 = mybir.dt.int32

N_CORES = 8
NT = 100          # tokens (neighbors) per graph
DIM = 512         # channel dim C
H = 4             # heads
HD = 128          # head dim
VOCAB = 100001    # emb rows
E = 256           # emb dim
SCALE = HD ** -0.5

AF = mybir.ActivationFunctionType
ALU = mybir.AluOpType
AX = mybir.AxisListType


def _bf16(a):
    return np.ascontiguousarray(a.astype(ml_dtypes.bfloat16))


def build_program(n_graphs, n_process=None, repeat=1):
    """Per-core program; all cores run it on different data.
    n_graphs must be a multiple of 32."""
    assert n_graphs % 32 == 0
    if n_process is None:
        n_process = n_graphs
    n_chunks = n_process // 8
    n_tail = n_graphs // 32           # batches of 32 graphs (128 rows)
    n_tail_run = n_process // 32

    nc = bass.Bass()

    emb = nc.declare_dram_parameter("emb", [VOCAB, E], BF16, False)
    idx = nc.declare_dram_parameter("idx", [NT, 2, n_graphs], I32, False)
    maskT = nc.declare_dram_parameter("maskT", [NT, n_graphs, NT], BF16, False)
    wqk = nc.declare_dram_parameter("wqk", [128, 4, 8, 128], BF16, False)
    gmat = nc.declare_dram_parameter("gmat", [128, 4, 4, DIM], BF16, False)
    fcT = nc.declare_dram_parameter("fcT", [128, 4, E], BF16, False)
    ident = nc.declare_dram_parameter("ident", [128, 128], BF16, False)
    out = nc.declare_dram_parameter("out", [n_tail, 128, E], F32, True)

    with tile.TileContext(nc) as tc:
        const = tc.alloc_tile_pool(name="const", bufs=1)
        xp = tc.alloc_tile_pool(name="xp", bufs=3)
        mp = tc.alloc_tile_pool(name="mp", bufs=3)
        sp = tc.alloc_tile_pool(name="sp", bufs=4)
        wp = tc.alloc_tile_pool(name="wp", bufs=2)
        up = tc.alloc_tile_pool(name="up", bufs=2)
        # PSUM: 8 banks x 2KB.  xnT 1 + qk 2 + lg 2 + sm 1 + wc 1 + tail 1.
        ps_a = tc.alloc_tile_pool(name="ps_a", bufs=1, space="PSUM")
        ps_qk = tc.alloc_tile_pool(name="ps_qk", bufs=2, space="PSUM")
        ps_lg = tc.alloc_tile_pool(name="ps_lg", bufs=2, space="PSUM")
        ps_sm = tc.alloc_tile_pool(name="ps_sm", bufs=1, space="PSUM")
        ps_wc = tc.alloc_tile_pool(name="ps_wc", bufs=1, space="PSUM")
        ps_tl = tc.alloc_tile_pool(name="ps_tl", bufs=1, space="PSUM")

        # ---- constants ----
        wqk_sb = const.tile([128, 4, 8, 128], BF16, tag="wqk")
        nc.sync.dma_start(out=wqk_sb[:], in_=wqk[:])
        g_sb = const.tile([128, 4, 4, DIM], BF16, tag="g_sb")
        nc.sync.dma_start(out=g_sb[:], in_=gmat[:])
        fcT_sb = const.tile([128, 4, E], BF16, tag="fcT")
        nc.sync.dma_start(out=fcT_sb[:], in_=fcT[:])
        ident_sb = const.tile([128, 128], BF16, tag="ident")
        nc.sync.dma_start(out=ident_sb[:], in_=ident[:])
        idx_sb = const.tile([NT, 2, n_graphs], I32, tag="idx")
        nc.sync.dma_start(out=idx_sb[:], in_=idx[:])
        eps_sb = const.tile([128, 1], F32, tag="eps")
        nc.vector.memset(eps_sb[:], 1e-5)
        wc_all = const.tile([128, n_tail, DIM], F32, tag="wc_all")
        nc.vector.memset(wc_all[:], 0.0)

        for chunk in range(n_chunks * repeat):
            chunk = chunk % n_chunks
            g0 = chunk * 8
            # gather x for 8 graphs: x[t, g, e, :] = emb[idx[t, g0+g, e]]
            x_sb = xp.tile([NT, 2, 8, E], BF16, tag="x")
            for ej in range(2):
                for g in range(8):
                    nc.gpsimd.indirect_dma_start(
                        out=x_sb[:, ej, g, :],
                        out_offset=None,
                        in_=emb[:, :],
                        in_offset=bass.IndirectOffsetOnAxis(
                            ap=idx_sb[:, ej, g0 + g:g0 + g + 1], axis=0),
                    )
            mask_sb = mp.tile([NT, 8, NT], BF16, tag="mask")
            nc.sync.dma_start(out=mask_sb[:], in_=maskT[:, g0:g0 + 8, :])

            # ---------- LN1 for the chunk ----------
            stats = sp.tile([NT, 8, 2, 6], F32, tag="stats")
            mv = sp.tile([NT, 8, 2], F32, tag="mv")
            for g in range(8):
                for sub in range(2):
                    nc.vector.bn_stats(out=stats[:, g, sub, :],
                                       in_=x_sb[:, sub, g, :])
                nc.vector.bn_aggr(out=mv[:, g, :], in_=stats[:, g, :, :])
            lnt = sp.tile([NT, 8], F32, tag="lnt")
            nc.scalar.activation(out=lnt[:], in_=mv[:, :, 1],
                                 func=AF.Ln, bias=eps_sb[:NT], scale=1.0)
            rs = sp.tile([NT, 8], F32, tag="rs")
            nc.scalar.activation(out=rs[:], in_=lnt[:], func=AF.Exp,
                                 scale=-0.5)
            xn_sb = xp.tile([NT, 8, DIM], BF16, tag="xn")
            for g in range(8):
                nc.vector.tensor_scalar(
                    out=xn_sb[:, g, :], in0=x_sb[:, :, g, :],
                    scalar1=mv[:, g, 0:1], scalar2=rs[:, g:g + 1],
                    op0=ALU.subtract, op1=ALU.mult)

            for pr in range(4):
                # ---------- transpose xn -> xnT [c, gl, kc, t] ----------
                xnT_ps = ps_a.tile([128, 2, 4, NT], BF16, tag="xnT",
                                   space="PSUM")
                for gl in range(2):
                    for kc in range(4):
                        nc.tensor.transpose(
                            out=xnT_ps[:, gl, kc, :],
                            in_=xn_sb[:, 2 * pr + gl,
                                      128 * kc:128 * (kc + 1)],
                            identity=ident_sb[:NT, :NT])
                xnT_sb = wp.tile([128, 2, 4, NT], BF16, tag="xnT")
                nc.vector.tensor_copy(out=xnT_sb[:, 0], in_=xnT_ps[:, 0])
                nc.scalar.activation(out=xnT_sb[:, 1], in_=xnT_ps[:, 1],
                                     func=AF.Copy)

                # ---------- q,k projections (pair-batched) ----------
                qk_sb = wp.tile([128, 8, 2, NT], BF16, tag="qk")
                for r in range(4):
                    qk_ps = ps_qk.tile([128, 2, 2, NT], F32, tag="qkps",
                                       space="PSUM")
                    for j in range(2):
                        for kc in range(4):
                            nc.tensor.matmul(
                                out=qk_ps[:, j, :, :],
                                lhsT=wqk_sb[:, kc, 2 * r + j, :],
                                rhs=xnT_sb[:, :, kc, :],
                                start=(kc == 0), stop=(kc == 3))
                    nc.scalar.activation(out=qk_sb[:, 2 * r:2 * r + 2, :, :],
                                         in_=qk_ps[:], func=AF.Copy)

                for gl in range(2):
                    g_loc = 2 * pr + gl
                    gg = g0 + g_loc       # graph index within core

                    # ---------- attention logits + masked exp ----------
                    lg_ps = ps_lg.tile([NT, H, NT], F32, tag="lg",
                                       space="PSUM")
                    for h in range(H):
                        nc.tensor.matmul(
                            out=lg_ps[:, h, :],
                            lhsT=qk_sb[:, h, gl, :],
                            rhs=qk_sb[:, 4 + h, gl, :],
                            start=True, stop=True)
                    e_sb = sp.tile([NT, H, NT], BF16, tag="e")
                    nc.scalar.activation(out=e_sb[:], in_=lg_ps[:],
                                         func=AF.Exp, scale=SCALE)
                    em_sb = sp.tile([NT, H, NT], BF16, tag="em")
                    dsum = sp.tile([NT, H], F32, tag="dsum")
                    for h in range(H):
                        nc.vector.tensor_tensor(
                            out=em_sb[:, h, :], in0=e_sb[:, h, :],
                            in1=mask_sb[:, g_loc, :], op=ALU.mult)
                    nc.vector.tensor_reduce(out=dsum[:], in_=em_sb[:],
                                            axis=AX.X, op=ALU.add)
                    r_f = sp.tile([NT, H], F32, tag="r_f")
                    nc.vector.reciprocal(out=r_f[:], in_=dsum[:])
                    r_sb = sp.tile([NT, H], BF16, tag="r")
                    nc.vector.tensor_copy(out=r_sb[:], in_=r_f[:])

                    # ---------- combine path (see module docstring) ------
                    sm_ps = ps_sm.tile([128, 84], F32, tag="sm",
                                       space="PSUM")
                    # amT[n,h] = sum_q em_h[q,n] * r_h[q]
                    for h in range(H):
                        nc.tensor.matmul(
                            out=sm_ps[:NT, h:h + 1],
                            lhsT=em_sb[:, h, :],
                            rhs=r_sb[:, h:h + 1],
                            start=True, stop=True)
                    # amT2[n,hp,h] = amT[n,h] * r_hp[n]
                    amT2 = sp.tile([NT, H, H], BF16, tag="amT2")
                    for hp in range(H):
                        nc.vector.tensor_scalar(
                            out=amT2[:, hp, :], in0=sm_ps[:NT, 0:4],
                            scalar1=r_f[:, hp:hp + 1], scalar2=None,
                            op0=ALU.mult)
                    # ZT[m,(hp,h)] = sum_n em_hp[n,m] * amT2[n,hp,h]
                    for hp in range(H):
                        nc.tensor.matmul(
                            out=sm_ps[:NT, 4 + 4 * hp:8 + 4 * hp],
                            lhsT=em_sb[:, hp, :],
                            rhs=amT2[:, hp, :],
                            start=True, stop=True)
                    zt_sb = sp.tile([NT, 16], BF16, tag="zt")
                    nc.scalar.activation(out=zt_sb[:], in_=sm_ps[:NT, 4:20],
                                         func=AF.Copy)
                    # UT[c,(hp,h)] = sum_m xn[m,c] * ZT[m,(hp,h)]
                    for kc in range(4):
                        nc.tensor.matmul(
                            out=sm_ps[:, 20 + 16 * kc:36 + 16 * kc],
                            lhsT=xn_sb[:, g_loc, 128 * kc:128 * (kc + 1)],
                            rhs=zt_sb[:],
                            start=True, stop=True)
                    if gg % 32 == 0:
                        ut_all = up.tile([128, 4, H, 32, H], BF16, tag="ut")
                    nc.scalar.activation(
                        out=ut_all[:, :, :, gg % 32, :],
                        in_=sm_ps[:, 20:84], func=AF.Copy)

            if (chunk + 1) % 4 == 0:
                # ---------- wc for batch of 32 graphs ----------
                t = chunk // 4
                wc_ps = ps_wc.tile([128, DIM], F32, tag="wc", space="PSUM")
                step = 0
                for hp in range(H):
                    for kc in range(4):
                        nc.tensor.matmul(
                            out=wc_ps[:],
                            lhsT=ut_all[:, kc, hp, :, :],
                            rhs=g_sb[:, kc, hp, :],
                            start=(step == 0), stop=(step == 15))
                        step += 1
                nc.scalar.activation(out=wc_all[:, t, :], in_=wc_ps[:],
                                     func=AF.Copy)

        # ---------- tail: LN2 + FC, batched 128 rows per batch ----------
        for t in range(n_tail_run):
            tst = sp.tile([128, 2, 6], F32, tag="tstats")
            tmv = sp.tile([128, 2], F32, tag="tmv")
            for sub in range(2):
                nc.vector.bn_stats(out=tst[:, sub, :],
                                   in_=wc_all[:, t, 256 * sub:256 * (sub + 1)])
            nc.vector.bn_aggr(out=tmv[:], in_=tst[:])
            tln = sp.tile([128, 1], F32, tag="tln")
            nc.scalar.activation(out=tln[:], in_=tmv[:, 1:2], func=AF.Ln,
                                 bias=eps_sb[:], scale=1.0)
            trs = sp.tile([128, 1], F32, tag="trs")
            nc.scalar.activation(out=trs[:], in_=tln[:], func=AF.Exp,
                                 scale=-0.5)
            wcn = sp.tile([128, DIM], BF16, tag="wcn")
            nc.vector.tensor_scalar(out=wcn[:], in0=wc_all[:, t, :],
                                    scalar1=tmv[:, 0:1], scalar2=trs[:],
                                    op0=ALU.subtract, op1=ALU.mult)
            wcnT_ps = ps_tl.tile([128, 4, 128], BF16, tag="tail",
                                 space="PSUM")
            for kc in range(4):
                nc.tensor.transpose(out=wcnT_ps[:, kc, :],
                                    in_=wcn[:, 128 * kc:128 * (kc + 1)],
                                    identity=ident_sb[:])
            wcnT = sp.tile([128, 4, 128], BF16, tag="wcnT")
            nc.vector.tensor_copy(out=wcnT[:], in_=wcnT_ps[:])
            o_ps = ps_tl.tile([128, E], F32, tag="tail", space="PSUM")
            for kc in range(4):
                nc.tensor.matmul(out=o_ps[:], lhsT=wcnT[:, kc, :],
                                 rhs=fcT_sb[:, kc, :],
                                 start=(kc == 0), stop=(kc == 3))
            o_sb = sp.tile([128, E], F32, tag="osb")
            nc.scalar.activation(out=o_sb[:], in_=o_ps[:], func=AF.Copy)
            nc.sync.dma_start(out=out[t], in_=o_sb[:])

        for _pool in (ps_tl, ps_wc, ps_sm, ps_lg, ps_qk, ps_a,
                      up, wp, sp, mp, xp, const):
            _pool.release()

    _split_matmul_waits(nc)
    return nc


_SPLIT_TYPES = (
    "InstMatmult", "InstLdweights", "InstTensorTensor", "InstTensorScalarPtr",
    "InstActivation", "InstTensorReduce", "InstTensorCopy", "InstBNStats",
    "InstBNStatsAggregate", "InstReciprocal", "InstTensorTensorReduce",
    "InstMemset", "InstDMACopy", "InstCopyPredicated", "InstSelect",
    "InstDrain",
)


def _split_matmul_waits(nc):
    """HW ISA slots hold a single sync-wait; move extras onto no-ops."""
    for fn in nc.m.functions:
        for blk in fn.blocks:
            new = []
            for inst in blk.instructions:
                si = getattr(inst, "sync_info", None)
                if (type(inst).__name__ in _SPLIT_TYPES
                        and si is not None and len(si.on_wait) > 1):
                    for w in si.on_wait[:-1]:
                        new.append(mybir.InstNoOp(
                            name=nc.get_next_instruction_name(),
                            engine=inst.engine,
                            bass_nofuse=True,
                            sync_info=mybir.SyncInfo(on_wait=[w],
                                                     on_update=[]),
                        ))
                    inst.sync_info = mybir.SyncInfo(
                        on_wait=[si.on_wait[-1]], on_update=si.on_update)
                new.append(inst)
            blk.instructions = new


def prep_host(inputs, n_graphs_total=1024, n_cores=N_CORES):
    """Fold LN params into weights, reshape/transpose inputs per core."""
    cons = np.asarray(inputs["connections"]).reshape(-1, NT, 2).astype(np.int32)
    mask = np.asarray(inputs["mask"]).reshape(-1, NT, NT).astype(np.float32)
    emb = np.asarray(inputs["emb"], dtype=np.float32)
    qkv_w = np.asarray(inputs["qkv_w"], dtype=np.float32)
    qkv_b = np.asarray(inputs["qkv_b"], dtype=np.float32)
    proj_w = np.asarray(inputs["proj_w"], dtype=np.float32)
    proj_b = np.asarray(inputs["proj_b"], dtype=np.float32)
    ln1_g = np.asarray(inputs["ln1_g"], dtype=np.float32)
    ln1_b = np.asarray(inputs["ln1_b"], dtype=np.float32)
    ln2_g = np.asarray(inputs["ln2_g"], dtype=np.float32)
    ln2_b = np.asarray(inputs["ln2_b"], dtype=np.float32)
    fc_w = np.asarray(inputs["fc_w"], dtype=np.float32)
    fc_b = np.asarray(inputs["fc_b"], dtype=np.float32)

    # fold LN1 gain/bias into qkv, LN2 gain/bias into fc (exact algebra)
    W = qkv_w * ln1_g[None, :]
    qb = qkv_b + qkv_w @ ln1_b
    Wf = fc_w * ln2_g[None, :]
    fb = fc_b + fc_w @ ln2_b
    assert not np.any(qb), "nonzero qkv bias not supported by this kernel"
    assert not np.any(proj_b), "nonzero proj bias not supported"
    assert not np.any(fb), "nonzero fc bias not supported"

    # lhsT tiles for q,k: [c_in_chunk(128), kc, oc, m]
    wqk = _bf16(W[:1024].reshape(8, 128, 4, 128).transpose(3, 2, 0, 1))
    # G_hp = Wv[hp_rows]^T @ proj_w[:, hp_cols]^T, rhs tiles
    # [c_in_within_chunk(128), kc, hp, c_out]
    Wv = W[1024:1536]
    G = np.stack([Wv[128 * hp:128 * (hp + 1), :].T
                  @ proj_w[:, 128 * hp:128 * (hp + 1)].T
                  for hp in range(4)])          # [hp, 512, 512]
    gmat = _bf16(G.reshape(4, 4, 128, DIM).transpose(2, 1, 0, 3))
    fcT = _bf16(Wf.T.reshape(4, 128, E).transpose(1, 0, 2))
    ident = _bf16(np.eye(128, dtype=np.float32))
    emb_bf = _bf16(emb)

    shared = {"wqk": wqk, "gmat": gmat, "fcT": fcT,
              "ident": ident, "emb": emb_bf}

    gpc = n_graphs_total // n_cores
    in_maps = []
    for c in range(n_cores):
        sl = slice(c * gpc, (c + 1) * gpc)
        m = dict(shared)
        m["idx"] = np.ascontiguousarray(cons[sl].transpose(1, 2, 0))
        m["maskT"] = _bf16(mask[sl].transpose(1, 0, 2))
        in_maps.append(m)
    return in_maps, gpc


_CACHE = {}


def kernel(**inputs):
    n_total = np.asarray(inputs["connections"]).reshape(-1, NT, 2).shape[0]
    in_maps, gpc = prep_host(inputs, n_total)
    if gpc not in _CACHE:
        _CACHE[gpc] = build_program(gpc)
    nc = _CACHE[gpc]
    res = run_bass_kernel_spmd(nc, in_maps, list(range(N_CORES)))
    outs = []
    for r in res.results:
        o = r["out"]          # [n_tail, 128, 256]; row p = 4*g_local + h
        nt = o.shape[0]
        o = o.reshape(nt * 32, 4, E)
        outs.append(o)
    return np.concatenate(outs, axis=0).astype(np.float32)
